# revision 1
# baseline (speedup 1.0000x reference)
"""EdgeGuidance Trainium2 kernel.

Pipeline per image [3,544,960] -> [1,136,240]:
  gray = w.RGB  ->  smooth = gauss5x5(reflect)  ->  gx,gy = sobel(zero-pad)
  mag = sqrt(gx^2+gy^2+1e-6)  ->  4x4 avgpool  ->  sigmoid(5(x-0.2))^2

All linear steps are folded into two banded-matrix passes on the PE:
  gx = A_x @ gray @ Bx^T,   gy = A_y @ gray @ By^T
where A_* = (vertical sobel, zero-pad) @ (vertical blur, reflect) and
B_* likewise horizontally. Phase A uses gray as the matmul stationary so
its output lands transposed ([w, s]); phase B then contracts over w with
the B^T band as stationary. float32r (TF32-like, full rate) throughout.

Data parallel over batch: 8 cores x 2 images.
"""

import numpy as np

import concourse.bass as bass
import concourse.tile as tile
from concourse import mybir
from concourse.bass_utils import run_bass_kernel_spmd

F32 = mybir.dt.float32
F32R = mybir.dt.float32r
AF = mybir.ActivationFunctionType
ALU = mybir.AluOpType

B_FULL, C, H, W = 16, 3, 544, 960
N_CORES = 8
B_LOC = B_FULL // N_CORES  # images per core
HP, WP = H // 4, W // 4  # 136, 240

BLUR_K, SIGMA = 5, 1.5
W_R, W_G, W_B = 0.2989, 0.587, 0.114

# s-blocks (240,240,64) and their gray row-blocks (K-blocks)
SB = [(0, 240), (240, 480), (480, 544)]
GB = [(0, 122), (122, 243), (237, 360), (360, 483), (477, 544)]
SB_GB = [[0, 1], [2, 3], [4]]  # gray-block indices per s-block
N_WC = 8  # w-chunks of 120 outputs each


def _wj(j):
    return max(0, 120 * j - 4), min(W, 120 * j + 124)


# ---------------------------------------------------------------- numpy bands
def _blur1d():
    x = np.arange(BLUR_K, dtype=np.float64) - (BLUR_K - 1) / 2.0
    g = np.exp(-(x**2) / (2.0 * SIGMA**2))
    return g / g.sum()


def _band_reflect(n, taps):
    r = len(taps) // 2
    m = np.zeros((n, n), dtype=np.float64)
    for s in range(n):
        for d in range(-r, r + 1):
            i = s + d
            if i < 0:
                i = -i
            elif i >= n:
                i = 2 * n - 2 - i
            m[s, i] += taps[d + r]
    return m


def _band_zero(n, taps):
    r = len(taps) // 2
    m = np.zeros((n, n), dtype=np.float64)
    for s in range(n):
        for d in range(-r, r + 1):
            i = s + d
            if 0 <= i < n:
                m[s, i] += taps[d + r]
    return m


def build_constants():
    g1 = _blur1d()
    vb_h = _band_reflect(H, g1)  # vertical blur on H
    hb_w = _band_reflect(W, g1)  # horizontal blur on W
    ax = _band_zero(H, [1.0, 2.0, 1.0]) @ vb_h
    ay = _band_zero(H, [-1.0, 0.0, 1.0]) @ vb_h
    bx = _band_zero(W, [-1.0, 0.0, 1.0]) @ hb_w
    by = _band_zero(W, [1.0, 2.0, 1.0]) @ hb_w
    # fold gray scale W_R into the vertical bands (gray' = R + aG + bB)
    ax *= W_R
    ay *= W_R

    band_a = np.zeros((128, 5 * 512), dtype=np.float32)
    v = 0
    for sb, (s0, s1) in enumerate(SB):
        for gb in SB_GB[sb]:
            r0, r1 = GB[gb]
            k, ns = r1 - r0, s1 - s0
            band_a[0:k, 512 * v + 0 : 512 * v + ns] = ax[s0:s1, r0:r1].T
            band_a[0:k, 512 * v + 256 : 512 * v + 256 + ns] = ay[s0:s1, r0:r1].T
            v += 1

    band_b = np.zeros((128, 2 * N_WC * 120), dtype=np.float32)
    for t, m in enumerate((bx, by)):
        for j in range(N_WC):
            w0, w1 = _wj(j)
            blk = m[120 * j : 120 * j + 120, w0:w1].T  # [Mj, 120]
            band_b[0 : w1 - w0, (t * N_WC + j) * 120 : (t * N_WC + j + 1) * 120] = blk

    p4 = np.zeros((128, 30), dtype=np.float32)
    for wp in range(120):
        p4[wp, wp // 4] = 1.0 / 16.0
    return band_a, band_b, p4


# ------------------------------------------------------------------ bass build
def split_multi_waits(nc):
    """walrus in this container only accepts 1 sync-wait per instruction;
    hoist extra waits onto preceding same-engine NoOps."""
    for fn in nc.m.functions:
        for bb in fn.blocks:
            new_list, changed = [], False
            for ins in bb.instructions:
                si = ins.sync_info
                waits = list(si.on_wait) if si is not None else []
                if len(waits) > 1:
                    changed = True
                    for i, wt in enumerate(waits[:-1]):
                        new_list.append(
                            mybir.InstNoOp(
                                name=f"{ins.name}_ws{i}",
                                engine=ins.engine,
                                bass_nofuse=True,
                                sync_info=mybir.SyncInfo(on_wait=[wt], on_update=[]),
                            )
                        )
                    si.on_wait = [waits[-1]]
                    ins.sync_info = si
                new_list.append(ins)
            if changed:
                bb.instructions = new_list


def build_module():
    nc = bass.Bass("TRN2", target_bir_lowering=False, debug=False)
    x = nc.dram_tensor("x", [B_LOC, C, H, W], F32, kind="ExternalInput").ap()
    ba = nc.dram_tensor("bA", [128, 5 * 512], F32, kind="ExternalInput").ap()
    bb_ = nc.dram_tensor("bB", [128, 2 * N_WC * 120], F32, kind="ExternalInput").ap()
    p4 = nc.dram_tensor("p4", [128, 30], F32, kind="ExternalInput").ap()
    y = nc.dram_tensor("y", [B_LOC, 1, HP, WP], F32, kind="ExternalOutput").ap()

    with tile.TileContext(nc) as tc:
        with (
            tc.tile_pool(name="const", bufs=1) as cpool,
            tc.tile_pool(name="rgb", bufs=4) as rgbp,
            tc.tile_pool(name="gray", bufs=10) as grayp,
            tc.tile_pool(name="xy", bufs=3) as xyp,
            tc.tile_pool(name="mag", bufs=3) as magp_,
            tc.tile_pool(name="outp", bufs=2) as outp,
            tc.tile_pool(name="psA", bufs=3, space="PSUM") as psA,
            tc.tile_pool(name="psB", bufs=1, space="PSUM") as psB,
            tc.tile_pool(name="psP", bufs=1, space="PSUM") as psP,
        ):
            # ---- constants: DMA in, round to f32r
            ba_raw = cpool.tile([128, 5 * 512], F32, tag="ba_raw")
            nc.sync.dma_start(ba_raw[:], ba[:])
            ba_t = cpool.tile([128, 5 * 512], F32, tag="ba")
            nc.vector.tensor_copy(ba_t[:].bitcast(F32R), ba_raw[:])

            bb_raw = cpool.tile([128, 2 * N_WC * 120], F32, tag="bb_raw")
            nc.sync.dma_start(bb_raw[:], bb_[:])
            bb_t = cpool.tile([128, 2 * N_WC * 120], F32, tag="bb")
            nc.vector.tensor_copy(bb_t[:].bitcast(F32R), bb_raw[:])

            p4_raw = cpool.tile([128, 30], F32, tag="p4_raw")
            nc.sync.dma_start(p4_raw[:], p4[:])
            p4_t = cpool.tile([128, 30], F32, tag="p4")
            nc.vector.tensor_copy(p4_t[:].bitcast(F32R), p4_raw[:])

            bias_eps = cpool.tile([128, 1], F32, tag="beps")
            nc.gpsimd.memset(bias_eps[:], 1e-6)
            bias_m1 = cpool.tile([128, 1], F32, tag="bm1")
            nc.gpsimd.memset(bias_m1[:], -1.0)

            for b in range(B_LOC):
                # ---- gray blocks (DVE): gray' = R + (wG/wR) G + (wB/wR) B
                gray_t = []
                for bi, (r0, r1) in enumerate(GB):
                    k = r1 - r0
                    rgb = rgbp.tile([128, 3 * W], F32, tag="rgb")
                    eng = nc.sync if bi % 2 == 0 else nc.scalar
                    eng.dma_start(
                        rgb[0:k, :].rearrange("p (c w) -> p c w", c=3),
                        x[b, :, r0:r1, :].rearrange("c r w -> r c w"),
                    )
                    tr = rgb[0:k, 0:W]
                    tg = rgb[0:k, W : 2 * W]
                    tb = rgb[0:k, 2 * W : 3 * W]
                    t1 = rgbp.tile([128, W], F32, tag="t1")
                    nc.vector.scalar_tensor_tensor(
                        t1[0:k, :], tg, W_G / W_R, tr,
                        op0=ALU.mult, op1=ALU.add,
                    )
                    gt = grayp.tile([128, W], F32, tag="gray")
                    nc.vector.scalar_tensor_tensor(
                        gt[0:k, :].bitcast(F32R), tb, W_B / W_R, t1[0:k, :],
                        op0=ALU.mult, op1=ALU.add,
                    )
                    gray_t.append(gt)

                pooled = psP.tile([128, 2 * WP], F32, tag="pooled")

                def stage_a(j):
                    # ---- phase A: xvT|yvT = (gray chunk)^T-contract bands
                    w0, w1 = _wj(j)
                    mj = w1 - w0
                    xy = xyp.tile([128, 3 * 512], F32, tag="xy")
                    for sb in range(3):
                        ps = psA.tile([128, 512], F32, tag="psA")
                        gbs = SB_GB[sb]
                        for i, gb in enumerate(gbs):
                            r0, r1 = GB[gb]
                            k = r1 - r0
                            nc.tensor.matmul(
                                ps[0:mj, :],
                                gray_t[gb][0:k, w0:w1].bitcast(F32R),
                                ba_t[0:k, 512 * gb : 512 * (gb + 1)].bitcast(F32R),
                                start=(i == 0),
                                stop=(i == len(gbs) - 1),
                            )
                        nc.scalar.copy(
                            xy[0:mj, 512 * sb : 512 * (sb + 1)].bitcast(F32R),
                            ps[0:mj, :],
                        )
                    return xy

                def stage_bc(j, xy):
                    w0, w1 = _wj(j)
                    mj = w1 - w0
                    # ---- phase B: gx/gy chunks [120, 768]
                    gxp = psB.tile([128, 768], F32, tag="gx")
                    gyp = psB.tile([128, 768], F32, tag="gy")
                    bxT = bb_t[0:mj, (0 * N_WC + j) * 120 : (0 * N_WC + j + 1) * 120]
                    byT = bb_t[0:mj, (1 * N_WC + j) * 120 : (1 * N_WC + j + 1) * 120]
                    xyv = xy[0:mj, :].rearrange("p (b c) -> p b c", b=3)
                    for t, (bT, gp) in enumerate(((bxT, gxp), (byT, gyp))):
                        nc.tensor.matmul(
                            gp[0:120, 0:512],
                            bT.bitcast(F32R),
                            xyv[:, 0:2, 256 * t : 256 * t + 256].bitcast(F32R),
                            start=True, stop=True,
                        )
                        nc.tensor.matmul(
                            gp[0:120, 512:768],
                            bT.bitcast(F32R),
                            xyv[:, 2, 256 * t : 256 * t + 256].bitcast(F32R),
                            start=True, stop=True,
                        )

                    # ---- mag = sqrt(gx^2 + gy^2 + 1e-6)
                    sqx = magp_.tile([128, 768], F32, tag="sqx")
                    nc.scalar.activation(sqx[0:120, :], gxp[0:120, :], AF.Square)
                    sqy = magp_.tile([128, 768], F32, tag="sqy")
                    nc.scalar.activation(sqy[0:120, :], gyp[0:120, :], AF.Square)
                    m2 = magp_.tile([128, 768], F32, tag="m2")
                    nc.vector.tensor_add(m2[0:120, :], sqx[0:120, :], sqy[0:120, :])
                    mg = magp_.tile([128, 768], F32, tag="mg")
                    nc.scalar.activation(
                        mg[0:120, :], m2[0:120, :], AF.Sqrt, bias=bias_eps[0:120, :]
                    )

                    # ---- s-pool (sum of 4 along s) -> [120, 192]
                    sp = magp_.tile([128, 192], F32, tag="sp")
                    with nc.allow_low_precision(reason="f32r is fp32-width"):
                        nc.vector.tensor_reduce(
                        sp[0:120, :].rearrange("p (b g) -> p b g", b=3, g=64)
                            .bitcast(F32R),
                            mg[0:120, :].rearrange(
                                "p (b g f) -> p b g f", b=3, g=64, f=4
                            ),
                            axis=mybir.AxisListType.X,
                            op=ALU.add,
                        )

                    # ---- w-pool via transpose-matmul -> pooled[s', 30j:30j+30]
                    for cih in range(2):
                        nc.tensor.matmul(
                            pooled[0:96, WP * cih + 30 * j : WP * cih + 30 * (j + 1)],
                            sp[0:120, 96 * cih : 96 * (cih + 1)].bitcast(F32R),
                            p4_t[0:120, :].bitcast(F32R),
                            start=True, stop=True,
                        )

                # software-pipelined driver: phase A runs one chunk ahead so
                # PE always has queued work while ACT/DVE drain chunk j-1
                xys = {}
                for j in range(N_WC + 1):
                    if j < N_WC:
                        xys[j] = stage_a(j)
                    if j >= 1:
                        stage_bc(j - 1, xys.pop(j - 1))

                # ---- sigmoid(5x-1)^2 on pooled [96, 480], then store
                sg = outp.tile([128, 2 * WP], F32, tag="sg")
                nc.scalar.activation(
                    sg[0:96, :], pooled[0:96, :], AF.Sigmoid,
                    bias=bias_m1[0:96, :], scale=5.0,
                )
                ot = outp.tile([128, 2 * WP], F32, tag="ot")
                nc.vector.tensor_mul(ot[0:96, :], sg[0:96, :], sg[0:96, :])
                # valid (partition-range, col-range) -> output s' rows
                nc.sync.dma_start(y[b, 0, 0:60, :], ot[0:60, 0:WP])
                nc.sync.dma_start(y[b, 0, 60:92, :], ot[64:96, 0:WP])
                nc.sync.dma_start(y[b, 0, 92:120, :], ot[0:28, WP : 2 * WP])
                nc.sync.dma_start(y[b, 0, 120:136, :], ot[32:48, WP : 2 * WP])

    split_multi_waits(nc)
    return nc


_NC = None
_CONSTS = None
TRACE = False
LAST_EXEC_NS = None


def kernel(**inputs):
    global _NC, _CONSTS, LAST_EXEC_NS
    left_rgb = np.ascontiguousarray(np.asarray(inputs["left_rgb"], dtype=np.float32))
    assert left_rgb.shape == (B_FULL, C, H, W)
    if _NC is None:
        _NC = build_module()
        _CONSTS = build_constants()
    band_a, band_b, p4 = _CONSTS
    in_maps = [
        {
            "x": np.ascontiguousarray(left_rgb[i * B_LOC : (i + 1) * B_LOC]),
            "bA": band_a,
            "bB": band_b,
            "p4": p4,
        }
        for i in range(N_CORES)
    ]
    res = run_bass_kernel_spmd(
        _NC, in_maps, core_ids=list(range(N_CORES)), trace=TRACE
    )
    LAST_EXEC_NS = res.exec_time_ns
    out = np.empty((B_FULL, 1, HP, WP), dtype=np.float32)
    for i in range(N_CORES):
        out[i * B_LOC : (i + 1) * B_LOC] = res.results[i]["y"]
    return out

